# revision 13
# baseline (speedup 1.0000x reference)
"""CNSDFM Trainium2 kernel: LSTM predictor + SDE/Kalman-style filter scan.

Data-parallel over batch: B=64 sharded across 8 NeuronCores (8 each).
Feature-major on-device layout: [feature(128 partitions), ..., batch].

Math notes (exact, no approximation):
- tanh(v) in the filter's drift net is computed as 1 - 2*r with
  r = 1/(1 + e^{2v}); the affine (1 - 2*r) is folded into the second-layer
  weights/bias: drift_out = (-2*W2) @ r + (b2 + W2 @ 1).  This keeps the
  whole filter scan in the single `natural_log_exp_and_others` ACT table set
  (softplus = ln(1 + e^x) needs exp+ln; no set has both ln and tanh).
- The noise-net input residual is computed negated ((pred+b)-z) and the sign
  is folded into noise_W1.
- x_{t+1} = u*(x_t + drift + softplus(psi)*n) + K*z with u = 1-K is computed
  as u*(x + d + sp*n + Kz/u): BKZ = Kz/u + drift_bias is precomputed in bulk
  and folded into the drift-psum preload, so the per-step tail is just
  x1 = (pout_d + x), m = sp*n, s = x1 + m, xn = u*s.  (u = 1-sigmoid(...) is
  bounded well away from 0 here, so Kz/u is numerically safe; verified by the
  end-to-end relative-error check.)
"""
import os
import sys
import numpy as np

for _p in ("/opt/trn_rl_repo", "/root/.axon_site/_ro/trn_rl_repo"):
    if os.path.isdir(_p) and _p not in sys.path:
        sys.path.insert(0, _p)

import concourse.bass as bass
import concourse.bacc as bacc
import concourse.mybir as mybir
import concourse.tile as tile
from concourse.bass_utils import run_bass_kernel_spmd
from concourse._compat import axon_active

AF = mybir.ActivationFunctionType
OP = mybir.AluOpType
F32 = mybir.dt.float32


# Constrain activation-table-set selection so the whole kernel needs exactly
# two ACT table loads (a reload is ~1.3us; the default greedy choice pairs
# Exp with exp_and_others and Ln with natural_log, reloading twice per filter
# step).  Names/indices are preserved; only membership is narrowed, so the
# emitted act_func_set_id still matches the compiler's act_info.json.
_orig_get_tables = None


def _patched_get_tables(arch):
    full = _orig_get_tables(arch)
    keep = {
        "sigmoid_and_others": {AF.Sigmoid, AF.Tanh, AF.Identity, AF.Relu},
        "natural_log_exp_and_others": {AF.Exp, AF.Ln, AF.Identity, AF.Relu},
    }
    return {name: (keep.get(name, set()) & fns if name in keep else set())
            for name, fns in full.items()}


def _install_table_patch():
    global _orig_get_tables
    import concourse.hw_specs as hw_specs
    if _orig_get_tables is None:
        _orig_get_tables = hw_specs.get_activation_tables
        bacc.get_activation_tables = _patched_get_tables

B, C, T_FULL, H = 64, 256, 512, 128
NCORES = 8
BL = B // NCORES          # 8 batch per core
CH = C // 128             # 2 feature chunks
GATE_PERM = [0, 1, 3, 2]  # torch (i,f,g,o) -> ours (i,f,o,g)


# --------------------------------------------------------------------------
# device program
# --------------------------------------------------------------------------
def build_nc(T):
    TF = T - 1  # filter steps
    _install_table_patch()
    nc = bacc.Bacc("TRN2", target_bir_lowering=False, debug=not axon_active(),
                   num_devices=NCORES)
    dram = {}

    def din(name, shape):
        dram[name] = nc.dram_tensor(name, shape, F32, kind="ExternalInput")
        return dram[name]

    # inputs (host-prepared layouts)
    zT = din("zT", [128, CH, T, BL])
    nT = din("nT", [128, CH, TF, BL])
    wih = din("wih", [128, CH, 4, 128])
    whh = din("whh", [128, 4, 128])
    ball = din("ball", [128, 4])
    fct = din("fct", [128, CH, 128])
    fcb = din("fcb", [128, CH])
    nw1m = din("nw1m", [128, CH, CH, 128])
    nb1 = din("nb1", [128, CH])
    nw2 = din("nw2", [128, CH, CH, 128])
    nb2 = din("nb2", [128, CH])
    dw1 = din("dw1", [128, CH, 128])
    db1x2 = din("db1x2", [128, 1])
    dw2m = din("dw2m", [128, CH, 128])
    fw1 = din("fw1", [128, CH, 128])
    fb1 = din("fb1", [128, 1])
    fw2 = din("fw2", [128, CH, 128])
    brep = din("brep", [128, 2 * CH, BL])  # pout preload bias (d_c0,d_c1,f_c0,f_c1)
    db2p = din("db2p", [128, CH])
    ident = din("ident", [128, 128])

    # outputs
    xs_o = nc.dram_tensor("xs", [128, CH, T, BL], F32, kind="ExternalOutput")
    u_o = nc.dram_tensor("u", [128, CH, T, BL], F32, kind="ExternalOutput")

    # internal DRAM scratch
    g_d = nc.dram_tensor("g_scr", [128, T, 4, BL], F32)
    bkz_d = nc.dram_tensor("bkz_scr", [128, CH, T, BL], F32)

    NT = T // 64          # 64-step tiles for bulk phases
    assert T % 64 == 0 and T % 16 == 0

    with tile.TileContext(nc) as tc:
        with (
            tc.tile_pool(name="wpool", bufs=1) as wp,
            tc.tile_pool(name="bigpool", bufs=1) as bp,
            tc.tile_pool(name="stream", bufs=2) as strm,
            tc.tile_pool(name="tmp", bufs=3) as tp,
        ):
            # ---- load weights/consts to SBUF ----
            sb = {}
            for name, hnd in dram.items():
                if name in ("zT", "nT"):
                    continue
                t_ = wp.tile(list(hnd.shape), F32, name=f"sb_{name}")
                nc.sync.dma_start(t_[:], hnd[:])
                sb[name] = t_
            zT_sb = bp.tile([128, CH, T, BL], F32, name="zT_sb")
            nc.sync.dma_start(zT_sb[:], zT[:])
            h_all = bp.tile([128, T + 1, BL], F32, name="h_all")
            xs_sb = bp.tile([128, CH, T, BL], F32, name="xs_sb")

            # PSUM pools, phase 1 (bulk1 + LSTM + bulk2): 6 of 8 banks
            from contextlib import ExitStack
            ps_ctx = ExitStack()
            psA = ps_ctx.enter_context(
                tc.tile_pool(name="psA", bufs=2, space="PSUM"))
            psB = ps_ctx.enter_context(
                tc.tile_pool(name="psB", bufs=2, space="PSUM"))
            psC = ps_ctx.enter_context(
                tc.tile_pool(name="psC", bufs=2, space="PSUM"))

            # ---- bulk1: G = Wih @ zT + ball  ->  g_d ----
            for nt in range(NT):
                stage = strm.tile([128, 64, 4, BL], F32, name="g_stage", tag="gstage")
                for g in range(4):
                    pg = psA.tile([128, 512], F32, name="pb1", tag="pb1")
                    for k in range(CH):
                        nc.tensor.matmul(
                            pg[:, :], sb["wih"][:, k, g, :],
                            zT_sb[:, k, nt * 64:(nt + 1) * 64, :],
                            start=(k == 0), stop=(k == CH - 1))
                    # psum -> staging with bias, strided (t,g,b) interleave
                    dst = stage[:, :, g, :]
                    if g % 2 == 0:
                        nc.vector.tensor_scalar(dst, pg[:, :].rearrange(
                            "p (t b) -> p t b", t=64), sb["ball"][:, g:g + 1],
                            None, OP.add)
                    else:
                        nc.scalar.activation(dst, pg[:, :].rearrange(
                            "p (t b) -> p t b", t=64), AF.Identity,
                            bias=sb["ball"][:, g:g + 1])
                nc.sync.dma_start(g_d[:, nt * 64:(nt + 1) * 64, :, :], stage[:])

            # ---- LSTM scan ----
            c_st = bp.tile([128, BL], F32, name="c_st")
            nc.vector.memset(c_st[:], 0.0)
            nc.vector.memset(h_all[:, 0, :], 0.0)

            for grp in range(T // 16):
                gld = strm.tile([128, 16, 4, BL], F32, name="g_ld", tag="gld")
                nc.sync.dma_start(gld[:], g_d[:, grp * 16:(grp + 1) * 16, :, :])
                pg = psB.tile([128, 512], F32, name="pgates", tag="pgates")
                nc.tensor.matmul(pg[:, :], sb["ident"][:],
                                 gld[:].rearrange("p t g b -> p (t g b)"),
                                 start=True, stop=False)
                for s in range(16):
                    t = grp * 16 + s
                    for g in range(4):
                        nc.tensor.matmul(
                            pg[:, s * 32 + g * 8: s * 32 + (g + 1) * 8],
                            sb["whh"][:, g, :], h_all[:, t, :],
                            start=False, stop=(s == 15 and g == 3))
                    sfo = tp.tile([128, 32], F32, name="sfo", tag="sfo")
                    nc.scalar.activation(sfo[:], pg[:, s * 32: s * 32 + 32],
                                         AF.Sigmoid)
                    nc.vector.tensor_tensor(c_st[:], c_st[:], sfo[:, 8:16], OP.mult)
                    gt = tp.tile([128, 8], F32, name="gt", tag="gt")
                    nc.vector.tensor_scalar(gt[:], sfo[:, 24:32], 2.0, -1.0,
                                            OP.mult, OP.add)
                    p1 = tp.tile([128, 8], F32, name="p1", tag="p1")
                    nc.vector.tensor_tensor(p1[:], sfo[:, 0:8], gt[:], OP.mult)
                    nc.vector.tensor_tensor(c_st[:], c_st[:], p1[:], OP.add)
                    th = tp.tile([128, 8], F32, name="th", tag="th")
                    nc.scalar.activation(th[:], c_st[:], AF.Tanh)
                    nc.vector.tensor_tensor(h_all[:, t + 1, :], sfo[:, 16:24],
                                            th[:], OP.mult)

            # ---- bulk2: pred, resid, K-net, u, kz; then UN ----
            for nt in range(NT):
                t0 = nt * 64
                rn = []
                for m in range(CH):
                    pp = psA.tile([128, 512], F32, name="ppred", tag="pb1")
                    nc.tensor.matmul(pp[:, :], sb["fct"][:, m, :],
                                     h_all[:, 1 + t0: 1 + t0 + 64, :],
                                     start=True, stop=True)
                    r_ = tp.tile([128, 64, BL], F32, name=f"rneg{m}", tag=f"rneg{m}")
                    nc.vector.scalar_tensor_tensor(
                        r_[:], pp[:, :].rearrange("p (t b) -> p t b", t=64),
                        sb["fcb"][:, m:m + 1], zT_sb[:, m, t0:t0 + 64, :],
                        OP.add, OP.subtract)
                    rn.append(r_)
                hr = []
                for m in range(CH):
                    ph = psB.tile([128, 512], F32, name="pkh", tag="pgates")
                    for k in range(CH):
                        nc.tensor.matmul(ph[:, :], sb["nw1m"][:, k, m, :],
                                         rn[k][:].rearrange("p t b -> p (t b)"),
                                         start=(k == 0), stop=(k == CH - 1))
                    h_ = tp.tile([128, 512], F32, name=f"hrelu{m}", tag=f"hrelu{m}")
                    nc.scalar.activation(h_[:], ph[:, :], AF.Relu,
                                         bias=sb["nb1"][:, m:m + 1])
                    hr.append(h_)
                for m in range(CH):
                    pk = psC.tile([128, 512], F32, name="pk", tag="pk")
                    for k in range(CH):
                        nc.tensor.matmul(pk[:, :], sb["nw2"][:, k, m, :],
                                         hr[k][:], start=(k == 0),
                                         stop=(k == CH - 1))
                    k_sb = tp.tile([128, 64, BL], F32, name="k_sb", tag="k_sb")
                    nc.scalar.activation(k_sb[:], pk[:, :].rearrange(
                        "p (t b) -> p t b", t=64), AF.Sigmoid,
                        bias=sb["nb2"][:, m:m + 1])
                    u_t = tp.tile([128, 64, BL], F32, name="u_t", tag="u_t")
                    nc.vector.tensor_scalar(u_t[:], k_sb[:], -1.0, 1.0,
                                            OP.mult, OP.add)
                    nc.sync.dma_start(u_o[:, m, t0:t0 + 64, :], u_t[:])
                    kz_t = tp.tile([128, 64, BL], F32, name="kz_t", tag="kz_t")
                    nc.vector.tensor_tensor(kz_t[:], k_sb[:],
                                            zT_sb[:, m, t0:t0 + 64, :], OP.mult)
                    if nt == 0 and m == 0:
                        # x0 = z[:, :, 0]
                        nc.vector.tensor_copy(xs_sb[:, :, 0, :],
                                              zT_sb[:, :, 0, :])
                    # bkz = kz/u + b2p (folded into the drift-psum preload;
                    # xn = u*(x + d + sp*n + kz/u) is exact)
                    ru_t = tp.tile([128, 64, BL], F32, name="ru_t", tag="ru_t")
                    nc.vector.reciprocal(ru_t[:], u_t[:])
                    bkz_t = tp.tile([128, 64, BL], F32, name="bkz_t", tag="bkz_t")
                    nc.vector.tensor_tensor(bkz_t[:], kz_t[:], ru_t[:], OP.mult)
                    nc.vector.tensor_scalar_add(bkz_t[:], bkz_t[:],
                                                sb["db2p"][:, m:m + 1])
                    nc.sync.dma_start(bkz_d[:, m, t0:t0 + 64, :], bkz_t[:])

            # ---- filter scan ----
            ps_ctx.close()
            ps_ctx2 = ExitStack()
            psH = ps_ctx2.enter_context(
                tc.tile_pool(name="psH", bufs=4, space="PSUM"))
            psO = ps_ctx2.enter_context(
                tc.tile_pool(name="psO", bufs=2, space="PSUM"))
            CHK = 64  # stream chunk (in filter steps t)
            n_chunks = (TF + CHK - 1) // CHK
            for ck in range(n_chunks):
                tlo = 1 + ck * CHK            # first t of chunk
                thi = min(tlo + CHK, T)       # exclusive
                ln_ = thi - tlo
                u_ld = strm.tile([128, CH, CHK, BL], F32, name="u_ld", tag="uld")
                bkz_ld = strm.tile([128, CH, CHK, BL], F32, name="bkz_ld", tag="bkzld")
                n_ld = strm.tile([128, CH, CHK, BL], F32, name="n_ld2", tag="nld2")
                nc.sync.dma_start(u_ld[:, :, 0:ln_, :], u_o[:, :, tlo:thi, :])
                nc.sync.dma_start(bkz_ld[:, :, 0:ln_, :], bkz_d[:, :, tlo:thi, :])
                nc.sync.dma_start(n_ld[:, :, 0:ln_, :],
                                  nT[:, :, tlo - 1:thi - 1, :])
                for s in range(ln_):
                    t = tlo + s
                    xp = [xs_sb[:, c_, t - 1, :] for c_ in range(CH)]
                    phd = psH.tile([128, 8], F32, name="phd", tag="ph")
                    nc.tensor.matmul(phd[:, :], sb["dw1"][:, 0, :], xp[0],
                                     start=True, stop=False)
                    nc.tensor.matmul(phd[:, :], sb["dw1"][:, 1, :], xp[1],
                                     start=False, stop=True)
                    phf = psH.tile([128, 8], F32, name="phf", tag="ph")
                    nc.tensor.matmul(phf[:, :], sb["fw1"][:, 0, :], xp[0],
                                     start=True, stop=False)
                    nc.tensor.matmul(phf[:, :], sb["fw1"][:, 1, :], xp[1],
                                     start=False, stop=True)
                    poutd = psO.tile([128, 2 * BL], F32, name="poutd", tag="poutd")
                    poutf = psO.tile([128, 2 * BL], F32, name="poutf", tag="poutf")
                    nc.tensor.matmul(poutd[:, :], sb["ident"][:],
                                     bkz_ld[:, :, s, :],
                                     start=True, stop=False)
                    nc.tensor.matmul(poutf[:, :], sb["ident"][:],
                                     sb["brep"][:, 2:4, :].rearrange("p r b -> p (r b)"),
                                     start=True, stop=False)
                    # drift: r = 1/(1+e^{2v+2b1}); out accumulates (-2W2)@r
                    ed = tp.tile([128, 8], F32, name="ed", tag="ed")
                    nc.scalar.activation(ed[:], phd[:, :], AF.Exp, scale=2.0,
                                         bias=sb["db1x2"][:, 0:1])
                    e1 = tp.tile([128, 8], F32, name="e1", tag="e1")
                    nc.vector.tensor_scalar(e1[:], ed[:], 1.0, None, OP.add)
                    rr = tp.tile([128, 8], F32, name="rr", tag="rr")
                    nc.vector.reciprocal(rr[:], e1[:])
                    # diff hidden: relu(v + b1) on DVE
                    rl = tp.tile([128, 8], F32, name="rl", tag="rl")
                    nc.vector.tensor_scalar(rl[:], phf[:, :], sb["fb1"][:, 0:1],
                                            0.0, OP.add, OP.max)
                    nc.tensor.matmul(poutd[:, 0:8], sb["dw2m"][:, 0, :], rr[:],
                                     start=False, stop=False)
                    nc.tensor.matmul(poutd[:, 8:16], sb["dw2m"][:, 1, :], rr[:],
                                     start=False, stop=True)
                    nc.tensor.matmul(poutf[:, 0:8], sb["fw2"][:, 0, :], rl[:],
                                     start=False, stop=False)
                    nc.tensor.matmul(poutf[:, 8:16], sb["fw2"][:, 1, :], rl[:],
                                     start=False, stop=True)
                    # softplus(diff) = ln(1 + e^psi)
                    ef = tp.tile([128, CH, 8], F32, name="ef", tag="ef")
                    nc.scalar.activation(
                        ef[:], poutf[:, :].rearrange("p (c b) -> p c b", c=2),
                        AF.Exp)
                    sp = tp.tile([128, CH, 8], F32, name="sp", tag="sp")
                    nc.scalar.activation(sp[:], ef[:], AF.Ln, bias=1.0)
                    # x1 = pout_d + x ; ux1 = u*x1 ; m = sp*UN ; y = ux1+m ; xn = y+kz
                    x1 = tp.tile([128, CH, 8], F32, name="x1", tag="x1")
                    nc.vector.tensor_tensor(
                        x1[:], poutd[:, :].rearrange("p (c b) -> p c b", c=2),
                        xs_sb[:, :, t - 1, :], OP.add)
                    m_ = tp.tile([128, CH, 8], F32, name="m_", tag="m_")
                    nc.vector.tensor_tensor(m_[:], sp[:], n_ld[:, :, s, :],
                                            OP.mult)
                    s_ = tp.tile([128, CH, 8], F32, name="s_", tag="s_")
                    nc.vector.tensor_tensor(s_[:], x1[:], m_[:], OP.add)
                    nc.vector.tensor_tensor(xs_sb[:, :, t, :], u_ld[:, :, s, :],
                                            s_[:], OP.mult)
                # dump xs chunk (covers t0 range incl x0 on first chunk)
                dlo = 0 if ck == 0 else tlo
                nc.sync.dma_start(xs_o[:, :, dlo:thi, :], xs_sb[:, :, dlo:thi, :])
            ps_ctx2.close()

    nc.compile()
    return nc


# --------------------------------------------------------------------------
# host-side input prep
# --------------------------------------------------------------------------
def prep_core_inputs(inputs, core, T):
    TF = T - 1
    s = slice(core * BL, (core + 1) * BL)
    f32 = lambda a: np.ascontiguousarray(a, dtype=np.float32)

    z = np.asarray(inputs["z"])[s, :, :T]            # [BL, C, T]
    noise = np.asarray(inputs["noise"])[:TF, s, :]   # [TF, BL, C]
    zT = z.reshape(BL, CH, 128, T).transpose(2, 1, 3, 0)
    nT = noise.reshape(TF, BL, CH, 128).transpose(3, 2, 0, 1)

    def blocks(w):  # [4H, X] -> gate-permuted blocks [4, H, X]
        return np.asarray(w).reshape(4, H, -1)[GATE_PERM]

    wih_b = blocks(inputs["lstm_Wih"]).copy()        # [4,128,256]
    wih_b[3] *= 2.0                                  # g-gate scaled: sigma(2g)
    wih = wih_b.reshape(4, 128, CH, 128).transpose(3, 2, 0, 1)  # [pk, kc, g, m]
    whh_b = blocks(inputs["lstm_Whh"]).copy()        # [4,128,128]
    whh_b[3] *= 2.0
    whh = whh_b.transpose(2, 0, 1)                   # [pk, g, m]
    ball_b = (np.asarray(inputs["lstm_bih"]) + np.asarray(inputs["lstm_bhh"]))
    ball = ball_b.reshape(4, H)[GATE_PERM].copy()
    ball[3] *= 2.0
    ball = ball.T                                    # [128, 4]

    fcW = np.asarray(inputs["fc_W"])                 # [C, H]
    fct = fcW.reshape(CH, 128, H).transpose(2, 0, 1)  # [ph, c, m]
    fcb = np.asarray(inputs["fc_b"]).reshape(CH, 128).T

    nW1 = np.asarray(inputs["noise_W1"])             # [C, C]
    nw1m = np.stack([[-nW1[mc * 128:(mc + 1) * 128, kc * 128:(kc + 1) * 128].T
                      for mc in range(CH)] for kc in range(CH)])  # [kc][mc][pk,m]
    nw1m = nw1m.transpose(2, 0, 1, 3)                # [pk, kc, mc, m]
    nb1 = np.asarray(inputs["noise_b1"]).reshape(CH, 128).T
    nW2 = np.asarray(inputs["noise_W2"])
    nw2 = np.stack([[nW2[mc * 128:(mc + 1) * 128, kc * 128:(kc + 1) * 128].T
                     for mc in range(CH)] for kc in range(CH)])
    nw2 = nw2.transpose(2, 0, 1, 3)
    nb2 = np.asarray(inputs["noise_b2"]).reshape(CH, 128).T

    dW1 = np.asarray(inputs["drift_W1"])             # [H, C]
    dw1 = dW1.reshape(H, CH, 128).transpose(2, 1, 0)  # [pk, kc, m]
    db1x2 = (2.0 * np.asarray(inputs["drift_b1"]))[:, None]
    dW2 = np.asarray(inputs["drift_W2"])             # [C, H]
    dw2m = (-2.0 * dW2).reshape(CH, 128, H).transpose(2, 0, 1)  # [ph, mc, m]
    db2p = (np.asarray(inputs["drift_b2"]) + dW2.sum(axis=1)).reshape(CH, 128).T

    fW1 = np.asarray(inputs["diff_W1"])
    fw1 = fW1.reshape(H, CH, 128).transpose(2, 1, 0)
    fb1 = np.asarray(inputs["diff_b1"])[:, None]
    fW2 = np.asarray(inputs["diff_W2"])
    fw2 = fW2.reshape(CH, 128, H).transpose(2, 0, 1)
    fb2 = np.asarray(inputs["diff_b2"]).reshape(CH, 128).T

    brep = np.stack([db2p[:, 0], db2p[:, 1], fb2[:, 0], fb2[:, 1]], axis=1)
    brep = np.repeat(brep[:, :, None], BL, axis=2)   # [128, 4, BL]

    return {k: f32(v) for k, v in dict(
        zT=zT, nT=nT, wih=wih, whh=whh, ball=ball, fct=fct, fcb=fcb,
        nw1m=nw1m, nb1=nb1, nw2=nw2, nb2=nb2, dw1=dw1, db1x2=db1x2,
        dw2m=dw2m, fw1=fw1, fb1=fb1, fw2=fw2, fb2=fb2, brep=brep,
        db2p=db2p, ident=np.eye(128)).items()}


_CACHE = {}


def _get_nc(T):
    if T not in _CACHE:
        _CACHE[T] = build_nc(T)
    return _CACHE[T]


def run_on_device(inputs, T=T_FULL, trace=False):
    nc = _get_nc(T)
    in_maps = [prep_core_inputs(inputs, c, T) for c in range(NCORES)]
    res = run_bass_kernel_spmd(nc, in_maps, core_ids=list(range(NCORES)),
                               trace=trace)
    return res


def assemble(res, inputs, T=T_FULL):
    z = np.asarray(inputs["z"])
    refined = np.empty((B, C, T), np.float32)
    uncert = np.empty((B, C, T), np.float32)
    for c in range(NCORES):
        s = slice(c * BL, (c + 1) * BL)
        xs = res.results[c]["xs"]        # [128, CH, T, BL]
        uu = res.results[c]["u"]
        refined[s] = xs.transpose(3, 1, 0, 2).reshape(BL, C, T)
        uncert[s] = uu.transpose(3, 1, 0, 2).reshape(BL, C, T)
    uncert[:, :, 0] = 0.0
    refined[:, :, 0] = z[:, :, 0]        # exact x0
    return refined, uncert


def kernel(**inputs):
    res = run_on_device(inputs, T_FULL)
    return assemble(res, inputs, T_FULL)
